# revision 21
# baseline (speedup 1.0000x reference)
"""GAT attention-score kernel for Trainium2 (8 NeuronCores, SPMD).

Computes e = LeakyReLU(Wx_i @ a[:D] + Wx_j @ a[D:], slope=0.2) for
E=640000 edges, D=128, sharded over 8 cores along the edge dimension
(a is replicated to every core).

Per-core layout (E_CORE = 80000 edges):
  - partition p owns edges [p*EPP, (p+1)*EPP) of the core's shard
  - T tiles of C edges/partition; each tile is one SBUF tensor
    [128, 2, C, 128] (Wx_i half + Wx_j half, one contiguous DMA each,
    split across the two HWDGE rings: SP for Wx_i, ACT for Wx_j)
  - the attention vector a is broadcast once to all partitions
    (a_sb [128, 2, 128])
  - per-tile compute = elementwise multiply by a (broadcast over edges)
    IN PLACE over the rec tile (stream read-before-write is safe within
    one instruction), then a per-edge reduction of the 256 products.
    No prod pool -> SBUF affords a 7-deep rec pipeline.  The two passes
    are spread over three engines via the tile plan:
      'W': VectorE mult, VectorE tensor_reduce(axis=XY)
      'A': VectorE mult, ScalarE activation(Copy, accum_out) per edge
      'B': GPSIMD  mult, ScalarE activation(Copy, accum_out) per edge
      'X': GPSIMD  mult, VectorE tensor_reduce(axis=XY)
  - LeakyReLU per tile on ScalarE (activation Lrelu) into a [128, EPP]
    result buffer; one store DMA at the end.
"""

import sys

if "/opt/trn_rl_repo" not in sys.path:
    sys.path.insert(0, "/opt/trn_rl_repo")

from contextlib import ExitStack

import numpy as np

import concourse.bass as bass
import concourse.bacc as bacc
import concourse.mybir as mybir
import concourse.tile as tile
from concourse.bass_utils import run_bass_kernel_spmd

N_CORES = 8
E = 640000
D = 128
REC = 2 * D
E_CORE = E // N_CORES  # 80000
P = 128
EPP = E_CORE // P  # 625 edges per partition
NEG_SLOPE = 0.2
F32 = mybir.dt.float32
MULT = mybir.AluOpType.mult
ADD = mybir.AluOpType.add
MAX = mybir.AluOpType.max

def make_plan(t_tiles: int, counts: dict[str, int] | None = None) -> str:
    """Largest-remainder interleave of tile kinds over t_tiles slots."""
    if counts is None:
        # engine budgets (per-tile costs: Vmult 6.8us, Vred 6.8, Gmult 14.4,
        # ACT accum+leaky 17.5): V=6.8*(2W+A+X), S=17.5*(A+B), G=14.4*(X+B)
        counts = {"W": 7, "X": 7, "A": 5, "B": 6}
    total = sum(counts.values())
    acc = {k: 0.0 for k in counts}
    out = []
    for _ in range(t_tiles):
        for k in counts:
            acc[k] += counts[k] / total
        k = max(acc, key=lambda q: acc[q])
        acc[k] -= 1.0
        out.append(k)
    return "".join(out)


TILE_PLAN = make_plan(25)


def _bcast_free(ap: bass.AP, count: int, axis: int) -> bass.AP:
    """Insert a stride-0 free dim of `count` at free-axis position `axis`."""
    dims = list(ap.ap)
    dims.insert(1 + axis, [0, count])
    return bass.AP(tensor=ap.tensor, offset=ap.offset, ap=dims)


def build_program(
    epp: int = EPP, c: int = 25, bufs: int = 7, plan: str | None = None
) -> bass.Bass:
    """Build the per-core Bass program for `epp` edges per partition."""
    assert epp % c == 0
    t_tiles = epp // c
    e_core = P * epp
    if plan is None:
        plan = make_plan(t_tiles)

    nc = bacc.Bacc()
    wi_d = nc.dram_tensor("Wx_i", [e_core, D], F32, kind="ExternalInput")
    wj_d = nc.dram_tensor("Wx_j", [e_core, D], F32, kind="ExternalInput")
    a_d = nc.dram_tensor("a", [REC], F32, kind="ExternalInput")
    out_d = nc.dram_tensor("out", [e_core], F32, kind="ExternalOutput")

    wi = wi_d[:].rearrange("(p n) d -> p n d", p=P)  # [128, epp, 128]
    wj = wj_d[:].rearrange("(p n) d -> p n d", p=P)
    out_r = out_d[:].rearrange("(p n) -> p n", p=P)  # [128, epp]

    with tile.TileContext(nc) as tc, ExitStack() as ctx:
        const_pool = ctx.enter_context(tc.tile_pool(name="const", bufs=1))
        in_pool = ctx.enter_context(tc.tile_pool(name="inp", bufs=bufs))
        acc_pool = ctx.enter_context(tc.tile_pool(name="acc", bufs=6))
        res_pool = ctx.enter_context(tc.tile_pool(name="res", bufs=1))

        # attention vector broadcast to all 128 partitions: [128, 2, 128]
        a_sb = const_pool.tile([P, 2, D], F32)
        a_ap = a_d[:]
        a_bcast = bass.AP(
            tensor=a_ap.tensor, offset=a_ap.offset, ap=[[0, P]] + list(a_ap.ap)
        )
        nc.gpsimd.dma_start(out=a_sb[:].rearrange("p a d -> p (a d)"), in_=a_bcast)

        # stride-0 garbage sink for the elementwise out of ScalarE accums
        sink_s = const_pool.tile([P, 1], F32)

        res = res_pool.tile([P, epp], F32)

        for t in range(t_tiles):
            kind = plan[t % len(plan)]
            rec = in_pool.tile([P, 2, c, D], F32, tag="rec")
            nc.sync.dma_start(out=rec[:, 0, :, :], in_=wi[:, t * c : (t + 1) * c, :])
            nc.scalar.dma_start(out=rec[:, 1, :, :], in_=wj[:, t * c : (t + 1) * c, :])

            acc = acc_pool.tile([P, c], F32, tag=f"acc_{kind}")

            # a_sb [P,2,D] viewed as [P,2,c,D] with stride-0 over c
            a_view = _bcast_free(a_sb[:], c, axis=1)
            eng = nc.gpsimd if kind in "BX" else nc.vector
            # in-place: prod overwrites rec
            eng.tensor_tensor(out=rec[:], in0=rec[:], in1=a_view, op=MULT)

            if kind in "WX":
                pv = rec[:].rearrange("p m c d -> p c m d")
                nc.vector.tensor_reduce(
                    out=acc[:], in_=pv, axis=mybir.AxisListType.XY, op=ADD
                )
            else:  # 'A' / 'B': ScalarE accumulates per edge
                for cc in range(c):
                    in_ = rec[:, :, cc, :]
                    nc.scalar.activation(
                        out=sink_s[:].broadcast_to(in_.shape),
                        in_=in_,
                        func=mybir.ActivationFunctionType.Copy,
                        accum_out=acc[:, cc : cc + 1],
                    )

            # leaky_relu on ScalarE, same engine as the A/B accumulators
            nc.scalar.activation(
                out=res[:, t * c : (t + 1) * c],
                in_=acc[:],
                func=mybir.ActivationFunctionType.Prelu,
                alpha=NEG_SLOPE,
            )

        nc.sync.dma_start(out=out_r[:, :], in_=res[:])

    nc.compile()
    return nc


_CACHED_NC = None


def kernel(Wx_i: np.ndarray, Wx_j: np.ndarray, a: np.ndarray) -> np.ndarray:
    global _CACHED_NC
    if _CACHED_NC is None:
        _CACHED_NC = build_program()
    nc = _CACHED_NC

    Wx_i = np.ascontiguousarray(np.asarray(Wx_i, dtype=np.float32))
    Wx_j = np.ascontiguousarray(np.asarray(Wx_j, dtype=np.float32))
    a = np.ascontiguousarray(np.asarray(a, dtype=np.float32))

    in_maps = []
    for i in range(N_CORES):
        sl = slice(i * E_CORE, (i + 1) * E_CORE)
        in_maps.append(
            {
                "Wx_i": np.ascontiguousarray(Wx_i[sl]),
                "Wx_j": np.ascontiguousarray(Wx_j[sl]),
                "a": a,
            }
        )

    r = run_bass_kernel_spmd(nc, in_maps, core_ids=list(range(N_CORES)))
    return np.concatenate([m["out"] for m in r.results])


# revision 22
# speedup vs baseline: 1.6209x; 1.6209x over previous
"""GAT attention-score kernel for Trainium2 (8 NeuronCores, SPMD).

Computes e = LeakyReLU(Wx_i @ a[:D] + Wx_j @ a[D:], slope=0.2) for
E=640000 edges, D=128, sharded over 8 cores along the edge dimension
(a is replicated to every core).

Per-core layout (E_CORE = 80000 edges):
  - partition p owns edges [p*EPP, (p+1)*EPP) of the core's shard
  - T tiles of C edges/partition; each tile is one SBUF tensor
    [128, 2, C, 128] (Wx_i half + Wx_j half, contiguous DMAs on the SP
    HWDGE ring); the attention vector a is broadcast once to all
    partitions (a_sb [128, 2, 128])
  - every tile is processed by two INDEPENDENT chains so no engine ever
    head-of-line blocks on another engine's stream:
      chain V (edges [0, QV)):   VectorE in-place mult by a, then
                                 VectorE tensor_reduce(axis=XY)
      chain G (edges [QV, C)):   GPSIMD in-place mult by a, then per
                                 edge one ScalarE activation(Copy,
                                 accum_out) free-axis sum
    VectorE depends only on the loads; GPSIMD only on the loads;
    ScalarE follows GPSIMD.
  - LeakyReLU (ScalarE Prelu, alpha=0.2) per chain into a [128, EPP]
    result buffer; one store DMA at the end.
"""

import sys

if "/opt/trn_rl_repo" not in sys.path:
    sys.path.insert(0, "/opt/trn_rl_repo")

from contextlib import ExitStack

import numpy as np

import concourse.bass as bass
import concourse.bacc as bacc
import concourse.mybir as mybir
import concourse.tile as tile
from concourse.bass_utils import run_bass_kernel_spmd

N_CORES = 8
E = 640000
D = 128
REC = 2 * D
E_CORE = E // N_CORES  # 80000
P = 128
EPP = E_CORE // P  # 625 edges per partition
NEG_SLOPE = 0.2
F32 = mybir.dt.float32
MULT = mybir.AluOpType.mult
ADD = mybir.AluOpType.add


def _bcast_free(ap: bass.AP, count: int, axis: int) -> bass.AP:
    """Insert a stride-0 free dim of `count` at free-axis position `axis`."""
    dims = list(ap.ap)
    dims.insert(1 + axis, [0, count])
    return bass.AP(tensor=ap.tensor, offset=ap.offset, ap=dims)


def build_program(
    epp: int = EPP, c: int = 25, bufs: int = 7, qv: int = 14
) -> bass.Bass:
    """Build the per-core Bass program for `epp` edges per partition.

    qv: edges per partition per tile handled by the VectorE chain;
        the remaining c-qv go through the GPSIMD->ScalarE chain.
    """
    assert epp % c == 0 and 0 < qv < c
    t_tiles = epp // c
    e_core = P * epp
    qa = c - qv

    nc = bacc.Bacc()
    wi_d = nc.dram_tensor("Wx_i", [e_core, D], F32, kind="ExternalInput")
    wj_d = nc.dram_tensor("Wx_j", [e_core, D], F32, kind="ExternalInput")
    a_d = nc.dram_tensor("a", [REC], F32, kind="ExternalInput")
    out_d = nc.dram_tensor("out", [e_core], F32, kind="ExternalOutput")

    wi = wi_d[:].rearrange("(p n) d -> p n d", p=P)  # [128, epp, 128]
    wj = wj_d[:].rearrange("(p n) d -> p n d", p=P)
    out_r = out_d[:].rearrange("(p n) -> p n", p=P)  # [128, epp]

    with tile.TileContext(nc) as tc, ExitStack() as ctx:
        const_pool = ctx.enter_context(tc.tile_pool(name="const", bufs=1))
        in_pool = ctx.enter_context(tc.tile_pool(name="inp", bufs=bufs))
        acc_pool = ctx.enter_context(tc.tile_pool(name="acc", bufs=6))
        res_pool = ctx.enter_context(tc.tile_pool(name="res", bufs=1))

        # attention vector broadcast to all 128 partitions: [128, 2, 128]
        a_sb = const_pool.tile([P, 2, D], F32)
        a_ap = a_d[:]
        a_bcast = bass.AP(
            tensor=a_ap.tensor, offset=a_ap.offset, ap=[[0, P]] + list(a_ap.ap)
        )
        nc.gpsimd.dma_start(out=a_sb[:].rearrange("p a d -> p (a d)"), in_=a_bcast)

        # stride-0 garbage sink for the elementwise out of ScalarE accums
        sink_s = const_pool.tile([P, 1], F32)

        res = res_pool.tile([P, epp], F32)

        for t in range(t_tiles):
            rec = in_pool.tile([P, 2, c, D], F32, tag="rec")
            nc.sync.dma_start(out=rec[:, 0, :, :], in_=wi[:, t * c : (t + 1) * c, :])
            nc.sync.dma_start(out=rec[:, 1, :, :], in_=wj[:, t * c : (t + 1) * c, :])

            # ---- chain V: VectorE mult + reduce over edges [0, qv)
            rv = rec[:, :, 0:qv, :]
            nc.vector.tensor_tensor(
                out=rv, in0=rv, in1=_bcast_free(a_sb[:], qv, axis=1), op=MULT
            )
            acc_v = acc_pool.tile([P, qv], F32, tag="acc_v")
            nc.vector.tensor_reduce(
                out=acc_v[:],
                in_=rv.rearrange("p m c d -> p c m d"),
                axis=mybir.AxisListType.XY,
                op=ADD,
            )

            # ---- chain G: GPSIMD mult, ScalarE accum over edges [qv, c)
            rg = rec[:, :, qv:c, :]
            nc.gpsimd.tensor_tensor(
                out=rg, in0=rg, in1=_bcast_free(a_sb[:], qa, axis=1), op=MULT
            )
            acc_a = acc_pool.tile([P, qa], F32, tag="acc_a")
            for cc in range(qa):
                in_ = rec[:, :, qv + cc, :]
                nc.scalar.activation(
                    out=sink_s[:].broadcast_to(in_.shape),
                    in_=in_,
                    func=mybir.ActivationFunctionType.Copy,
                    accum_out=acc_a[:, cc : cc + 1],
                )

            # leaky relus on ScalarE into the result buffer
            nc.scalar.activation(
                out=res[:, t * c : t * c + qv],
                in_=acc_v[:],
                func=mybir.ActivationFunctionType.Prelu,
                alpha=NEG_SLOPE,
            )
            nc.scalar.activation(
                out=res[:, t * c + qv : (t + 1) * c],
                in_=acc_a[:],
                func=mybir.ActivationFunctionType.Prelu,
                alpha=NEG_SLOPE,
            )

        nc.scalar.dma_start(out=out_r[:, :], in_=res[:])

    nc.compile()
    return nc


_CACHED_NC = None


def kernel(Wx_i: np.ndarray, Wx_j: np.ndarray, a: np.ndarray) -> np.ndarray:
    global _CACHED_NC
    if _CACHED_NC is None:
        _CACHED_NC = build_program()
    nc = _CACHED_NC

    Wx_i = np.ascontiguousarray(np.asarray(Wx_i, dtype=np.float32))
    Wx_j = np.ascontiguousarray(np.asarray(Wx_j, dtype=np.float32))
    a = np.ascontiguousarray(np.asarray(a, dtype=np.float32))

    in_maps = []
    for i in range(N_CORES):
        sl = slice(i * E_CORE, (i + 1) * E_CORE)
        in_maps.append(
            {
                "Wx_i": np.ascontiguousarray(Wx_i[sl]),
                "Wx_j": np.ascontiguousarray(Wx_j[sl]),
                "a": a,
            }
        )

    r = run_bass_kernel_spmd(nc, in_maps, core_ids=list(range(N_CORES)))
    return np.concatenate([m["out"] for m in r.results])
